# revision 28
# baseline (speedup 1.0000x reference)
"""Trainium2 (8-core SPMD) kernel for the ActorCriticTensorNet MPS head.

reference:
    env0 = einsum('e,eoij->oij', x[0], mps[0])
    for a in 1..63: env = sigmoid(env @ einsum('e,eoij->oij', x[a], mps[a]))
    out = einsum('oii->o', env)

Strategy: the computation factorizes perfectly over the output channel o —
the per-agent contractions mat[a][o] and the 63-step sigmoid chain for
channel o never touch any other channel; the channels only meet in the final
trace vector.  So shard by o: core c receives mps[:, :, c, :, :] (16.8 MB in
fp8) plus the full (tiny) x, computes all 64 mat[g][c] slices locally, runs
its own o=c chain locally, and ships its final 32x32 env; the host takes the
8 traces.  Zero inter-core communication.

Phase-1 layout (per agent): weight column (gi, eh, c) is the contiguous run
packed[(gi*2 + eh)*1024 + c*128 : +128] with per-column packing
p = 32*jh + k (k = chain row, jh = j>>3, c = j&7), so
psum[32*jh + k, gi*8 + c] = mat[gi][k, 8*jh + c] after the two-eh
accumulation.  Four 32-partition DVE copies (cross-quadrant moves are free)
then drop each psum quadrant jh straight into the chain-weight slab
cw[k, gi*32 + 8*jh + c] = mat[gi][k, j] in SBUF — no DRAM round trip.

Phase-1 operands are fp8 e3m4 with power-of-two scales (mps x128, x x2 —
both mid-range in e3m4's +-15.5), halving HBM traffic vs bf16 and enabling
4x fast-weight-load (LDWEIGHTS ~27 ns per 128x128 block).  PSUM accumulates
the 256x-scaled mats in fp32; the 1/256 descale folds into the chain
sigmoid's scale parameter.  The chain runs in bf16 (measured end-to-end
relative error ~1.4e-3; gate is 2e-2).

The chain maintains env TRANSPOSED (stored' = mat^T stored), so agent 0's
slab is packed pre-transposed host-side and serves directly as env0 — no
identity matrix, no PE transpose, no ACT copy.  Step 1 then carries a
double 256x scale (both operands are 256x-scaled slabs) that folds into
its sigmoid scale as 1/65536.  Dropping the ACT copy also makes the
sigmoid table load the first ACT-queue instruction, so it overlaps the
framework preamble instead of blocking chain step 1.

Scheduling: the 63-step sigmoid chain is strictly serial at ~600 ns/step
(MM ~180 + sem + ACT ~280 + sem), so it must fully overlap the mps DMA
stream (16.8 MB at ~400 GB/s = ~42 us — the roofline) and phase 1.  The
Tile scheduler ignores program order, so the PE stream is pinned with
add_dep_helper(sync=True) edges into strict alternation
[agent block][one chain step]: a single chain MM rides in the PE pipeline
shadow of each block, and its sigmoid retires while the next block
streams (~600-660 ns/agent).  Two chain MMs in one slot would stall the
next block a full sigmoid round-trip (PE dispatch is in-order), so
catch-up bursts are rate-limited to one per 6 slots, and only when the
chain is >4 steps behind.  Segments are 2 agents (chain lags ~2 agents
behind phase-1) while DMA chunks are 2 MB mid-stream for line rate,
tapered at both ends; the 2 MB chunks recycle through 3 buffers so at
most ~2 transfers are outstanding — a long queue of outstanding DMAs
was measured to slow early-chunk completion by 1-4 us (bandwidth
sharing), delaying chain start.
"""

import numpy as np

A, E, O, C = 64, 256, 8, 32
FO = C * C  # per-o mat size: 1024
N_CORES = 8
# Compute segments: 2 agents each (the relayout granularity).  DMA chunks
# coincide 1:1 with segments: 512 KB transfers, explicitly chained 3-deep
# so at most ~3 are outstanding — the SDMA engines round-robin across all
# queued transfers at packet granularity, so a long queue delays every
# individual completion (measured 1-4 us on early chunks) even though the
# aggregate rate stays at line rate.  Completion then tracks the stream
# position and agent g's weights land ~1 us after its bytes pass.
SEGS = [(0, 1), (1, 1)] + [(2 + 2 * b, 2) for b in range(31)]
DMA_CHUNKS = [(0, 2), (2, 2), (4, 2), (6, 2)] + [
    (8 + 4 * b, 4) for b in range(14)
]

_CACHE = {}


def _build():
    from concourse import bacc, mybir, tile
    from concourse.tile_rust import add_dep_helper

    F32 = mybir.dt.float32
    BF16 = mybir.dt.bfloat16
    FP8 = mybir.dt.float8e3
    SIG = mybir.ActivationFunctionType.Sigmoid
    nc = bacc.Bacc(
        "TRN2", target_bir_lowering=False, debug=False, num_devices=N_CORES
    )
    x_d = nc.dram_tensor("inputs", [128, 2 * A], FP8, kind="ExternalInput")
    mps_d = nc.dram_tensor(
        "mps", [128, A * 2 * FO], FP8, kind="ExternalInput"
    )
    out_d = nc.dram_tensor("out", [C, C], BF16, kind="ExternalOutput")

    with tile.TileContext(nc) as tc:
        with (
            # head/tail chunks get dedicated buffers; the 2 MB mid-stream
            # chunks recycle through 3 buffers (WAR on the last phase-1
            # read re-gates dispatch, capping outstanding transfers at ~2)
            tc.tile_pool(name="mps2", bufs=4) as mps2,
            tc.tile_pool(name="mps4", bufs=14) as mps4,
            tc.tile_pool(name="small", bufs=1) as small,
            tc.tile_pool(name="cw_pool", bufs=8) as cw_pool,
            tc.tile_pool(name="env_pool", bufs=4) as env_pool,
            tc.tile_pool(name="ps_mat", bufs=4, space="PSUM") as ps_mat,
            tc.tile_pool(name="ps_chain", bufs=3, space="PSUM") as ps_chain,
        ):
            seg_of = {}
            for si, (g0, w) in enumerate(SEGS):
                for g in range(g0, g0 + w):
                    seg_of[g] = si

            # all input DMAs dispatch up front on the sync HWDGE ring
            x_sb = small.tile([128, 2 * A], FP8)
            nc.sync.dma_start(x_sb[:], x_d[:])
            chunk_views = []  # (c0, cw, view)
            chunk_dmas = []
            for c0, cw in DMA_CHUNKS:
                pool = {2: mps2, 4: mps4}[cw]
                t = pool.tile([128, cw * 2 * FO], FP8, tag=f"mps{cw}")
                dma = nc.sync.dma_start(
                    t[:], mps_d[:, c0 * 2 * FO : (c0 + cw) * 2 * FO]
                )
                if len(chunk_dmas) >= 8:
                    # cap outstanding transfers: chunk k dispatches only
                    # after chunk k-8 has fully landed
                    add_dep_helper(
                        dma.ins,
                        chunk_dmas[-8].ins,
                        sync=True,
                        reason="limit outstanding DMA to 8",
                    )
                chunk_dmas.append(dma)
                chunk_views.append(
                    (
                        c0,
                        cw,
                        t[:].rearrange(
                            "e (gi eh c p) -> e gi eh c p",
                            gi=cw,
                            eh=2,
                            c=8,
                            p=128,
                        ),
                    )
                )

            def seg_view(g0, w):
                # slice the covering DMA chunk down to this segment
                for c0, cw, v in chunk_views:
                    if c0 <= g0 and g0 + w <= c0 + cw:
                        return v[:, g0 - c0 : g0 - c0 + w]
                raise AssertionError("segment not covered by a chunk")

            envs = [None]
            cvs = {}

            def chain_step(g, after=None):
                si = seg_of[g]
                cv, gi = cvs[si], g - SEGS[si][0]
                ps_g = ps_chain.tile([C, C], F32, tag="cps")
                pe = nc.tensor.matmul(
                    ps_g[:],
                    cv[:, gi, :],
                    envs[0],
                    start=True,
                    stop=True,
                )
                if after is not None:
                    # pin the chain matmul after the current agent block so
                    # the scheduler cannot bunch chain steps early either
                    add_dep_helper(
                        pe.ins,
                        after.ins,
                        sync=True,
                        reason="pace chain step to its agent slot",
                    )
                env2 = env_pool.tile([C, C], BF16, tag="env")
                # operands carry 256x fp8 scales (mps x128, x x2); the
                # descale folds into the sigmoid's scale.  Step 1's moving
                # operand is agent 0's raw 256x slab, hence 1/65536.
                nc.scalar.activation(
                    env2[:],
                    ps_g[:],
                    SIG,
                    scale=1.0 / 65536.0 if g == 1 else 1.0 / 256.0,
                )
                envs[0] = env2[:]
                return pe

            next_chain = 1
            last_chain = [None]
            slot = [0]
            last_burst = [-10]

            for si, (g0, w) in enumerate(SEGS):
                psum_b = ps_mat.tile([128, 8 * w], F32, tag="psa")
                tvs = seg_view(g0, w)
                for gi in range(w):
                    g = g0 + gi
                    last_mm = None
                    for c in range(8):
                        for eh in range(2):
                            mm = nc.tensor.matmul(
                                psum_b[:, gi * 8 + c : gi * 8 + c + 1],
                                tvs[:, gi, eh, c, :],
                                x_sb[:, eh * A + g : eh * A + g + 1],
                                start=(eh == 0),
                                stop=(eh == 1),
                            )
                            last_mm = mm
                            if last_chain[0] is not None:
                                # pin EVERY phase-1 matmul after the latest
                                # chain matmul (sync=True: no_sync edges
                                # are ignored for ordering) so phase-1
                                # cannot run ahead of the chain; with the
                                # reverse pin in chain_step the PE stream
                                # strictly alternates [agent block][chain
                                # step] and each sigmoid retires while the
                                # next agent block streams
                                add_dep_helper(
                                    mm.ins,
                                    last_chain[0].ins,
                                    sync=True,
                                    reason="interleave chain with phase-1",
                                )
                    # one chain step per agent block: the chain MM rides
                    # in the block's PE pipeline shadow.  A second step in
                    # one slot stalls the next block a full sigmoid
                    # round-trip, so catch-up bursts are rate-limited.
                    # elig = g - 2: chain t fires two block-cycles after
                    # its segment's relayout began, so the 4 serial DVE
                    # copies (~0.96 us end-to-end) are never on the
                    # chain's critical path.
                    slot[0] += 1
                    # head: 1-agent lag (the early CASTs retire during DMA
                    # waits, so the tighter slot is stall-free and starts
                    # the chain one block earlier); steady state: 2-agent
                    # lag keeps the per-segment relayout off the chain path
                    elig = g - 1 if g < 8 else g - 2
                    backlog = elig - next_chain
                    cap = 1
                    if backlog > 4 and slot[0] - last_burst[0] >= 6:
                        cap = 2
                        last_burst[0] = slot[0]
                    k = 0
                    while next_chain < elig and k < cap:
                        last_chain[0] = chain_step(next_chain, after=last_mm)
                        next_chain += 1
                        k += 1
                # psum[32*jh + k, gi*8 + c] -> cw[k, gi*32 + jh*8 + c]:
                # one cross-quadrant DVE copy per psum quadrant jh.
                cw = cw_pool.tile([C, 32 * w], BF16, tag="cw", name=f"cw{si}")
                cwv = cw[:].rearrange(
                    "k (gi jh c) -> k gi jh c", gi=w, jh=4, c=8
                )
                for jh in range(4):
                    nc.vector.tensor_copy(
                        cwv[:, :, jh, :],
                        psum_b[32 * jh : 32 * jh + 32, :].rearrange(
                            "k (gi c) -> k gi c", gi=w, c=8
                        ),
                    )
                cvs[si] = cw[:].rearrange("k (gi j) -> k gi j", gi=w, j=32)
                if si == 0:
                    # agent 0's slab is packed pre-transposed host-side:
                    # it IS env0 (times 256), read directly as the moving
                    # operand of chain step 1.
                    envs[0] = cvs[0][:, 0, :]
            while next_chain < A:
                chain_step(next_chain)
                next_chain += 1

            # ship the final 32x32 env; host takes the trace.  Issued on
            # the ACT engine's HWDGE ring: its queue is idle right after
            # the last sigmoid, skipping a cross-engine sem hop.
            nc.scalar.dma_start(out_d[:], envs[0])

    nc.compile()
    return nc


def get_nc():
    if "nc" not in _CACHE:
        _CACHE["nc"] = _build()
    return _CACHE["nc"]


def make_in_maps(inputs, mps):
    import ml_dtypes

    FP8 = ml_dtypes.float8_e3m4
    # power-of-two scales put both operands mid-range in e3m4 (max 15.5):
    # x ~ N(0,1) * 2, mps ~ N(0, 0.0156^2) * 128.  Combined 256x descales
    # on-device via the chain sigmoid's scale parameter.
    x = (np.asarray(inputs, dtype=np.float32) * 2.0).astype(FP8)
    mps = np.asarray(mps, dtype=np.float32).reshape(A, E, O, FO)
    # x packed as [e_low, (e_chunk, agent)]
    x_pack = np.ascontiguousarray(
        x.reshape(A, 2, 128).transpose(2, 1, 0).reshape(128, 2 * A)
    )
    # F_idx[c, p] = k*32 + j with k = p%32, j = 8*(p//32) + c: weight column
    # (gi, eh, c) reads the contiguous run packed[(gi*2+eh)*1024 + c*128 :
    # +128] and psum partitions come out as 32*jh + k (chain-quadrant
    # layout).  Agent 0 swaps k and j (transposed slab = env0 directly).
    p = np.arange(128)
    c = np.arange(8)[:, None]
    F_idx = ((p % 32) * 32 + (p // 32) * 8 + c).reshape(-1)  # (1024,)
    F_idx0 = (((p // 32) * 8 + c) * 32 + (p % 32)).reshape(-1)
    in_maps = []
    for ci in range(N_CORES):
        m = (mps[:, :, ci, :] * 128.0).astype(FP8)  # (A, E, FO)
        m = m[:, :, F_idx]  # permute f so weight columns are contiguous
        m[0] = (mps[0, :, ci, :] * 128.0).astype(FP8)[:, F_idx0]
        # -> [e_low(128), agent, e_chunk, FO]  (one contiguous run per
        #    chunk: both e-halves of its agents)
        m = m.reshape(A, 2, 128, FO).transpose(2, 0, 1, 3)
        in_maps.append(
            {
                "inputs": x_pack,
                "mps": np.ascontiguousarray(m).reshape(128, A * 2 * FO),
            }
        )
    return in_maps


def kernel(inputs, mps):
    from concourse.bass_utils import run_bass_kernel_spmd

    nc = get_nc()
    in_maps = make_in_maps(inputs, mps)
    try:
        res = run_bass_kernel_spmd(nc, in_maps, core_ids=list(range(N_CORES)))
    except Exception:
        # rare transient NRT failures; one retry
        res = run_bass_kernel_spmd(nc, in_maps, core_ids=list(range(N_CORES)))
    return np.array(
        [
            np.trace(res.results[ci]["out"].astype(np.float32))
            for ci in range(N_CORES)
        ],
        dtype=np.float32,
    )


# revision 29
# speedup vs baseline: 1.0370x; 1.0370x over previous
"""Trainium2 (8-core SPMD) kernel for the ActorCriticTensorNet MPS head.

reference:
    env0 = einsum('e,eoij->oij', x[0], mps[0])
    for a in 1..63: env = sigmoid(env @ einsum('e,eoij->oij', x[a], mps[a]))
    out = einsum('oii->o', env)

Strategy: the computation factorizes perfectly over the output channel o —
the per-agent contractions mat[a][o] and the 63-step sigmoid chain for
channel o never touch any other channel; the channels only meet in the final
trace vector.  So shard by o: core c receives mps[:, :, c, :, :] (16.8 MB in
fp8) plus the full (tiny) x, computes all 64 mat[g][c] slices locally, runs
its own o=c chain locally, and ships its final 32x32 env; the host takes the
8 traces.  Zero inter-core communication.

Phase-1 layout (per agent): weight column (gi, eh, c) is the contiguous run
packed[(gi*2 + eh)*1024 + c*128 : +128] with per-column packing
p = 32*jh + k (k = chain row, jh = j>>3, c = j&7), so
psum[32*jh + k, gi*8 + c] = mat[gi][k, 8*jh + c] after the two-eh
accumulation.  Four 32-partition DVE copies (cross-quadrant moves are free)
then drop each psum quadrant jh straight into the chain-weight slab
cw[k, gi*32 + 8*jh + c] = mat[gi][k, j] in SBUF — no DRAM round trip.

Phase-1 operands are fp8 e3m4 with power-of-two scales (mps x128, x x2 —
both mid-range in e3m4's +-15.5), halving HBM traffic vs bf16 and enabling
4x fast-weight-load (LDWEIGHTS ~27 ns per 128x128 block).  PSUM accumulates
the 256x-scaled mats in fp32; the 1/256 descale folds into the chain
sigmoid's scale parameter.  The chain runs in bf16 (measured end-to-end
relative error ~1.4e-3; gate is 2e-2).

The chain maintains env TRANSPOSED (stored' = mat^T stored), so agent 0's
slab is packed pre-transposed host-side and serves directly as env0 — no
identity matrix, no PE transpose, no ACT copy.  Step 1 then carries a
double 256x scale (both operands are 256x-scaled slabs) that folds into
its sigmoid scale as 1/65536.  Dropping the ACT copy also makes the
sigmoid table load the first ACT-queue instruction, so it overlaps the
framework preamble instead of blocking chain step 1.

Scheduling: the 63-step sigmoid chain is strictly serial at ~600 ns/step
(MM ~180 + sem + ACT ~280 + sem), so it must fully overlap the mps DMA
stream (16.8 MB at ~400 GB/s = ~42 us — the roofline) and phase 1.  The
Tile scheduler ignores program order, so the PE stream is pinned with
add_dep_helper(sync=True) edges into strict alternation
[agent block][one chain step]: a single chain MM rides in the PE pipeline
shadow of each block, and its sigmoid retires while the next block
streams (~600-660 ns/agent).  Two chain MMs in one slot would stall the
next block a full sigmoid round-trip (PE dispatch is in-order), so
catch-up bursts are rate-limited to one per 6 slots, and only when the
chain is >4 steps behind.  Segments are 2 agents (chain lags ~2 agents
behind phase-1) while DMA chunks are 2 MB mid-stream for line rate,
tapered at both ends; the 2 MB chunks recycle through 3 buffers so at
most ~2 transfers are outstanding — a long queue of outstanding DMAs
was measured to slow early-chunk completion by 1-4 us (bandwidth
sharing), delaying chain start.
"""

import numpy as np

A, E, O, C = 64, 256, 8, 32
FO = C * C  # per-o mat size: 1024
N_CORES = 8
# Compute segments: 2 agents each (the relayout granularity).  DMA chunks
# coincide 1:1 with segments: 512 KB transfers, explicitly chained 3-deep
# so at most ~3 are outstanding — the SDMA engines round-robin across all
# queued transfers at packet granularity, so a long queue delays every
# individual completion (measured 1-4 us on early chunks) even though the
# aggregate rate stays at line rate.  Completion then tracks the stream
# position and agent g's weights land ~1 us after its bytes pass.
SEGS = [(0, 1), (1, 1)] + [(2 + 2 * b, 2) for b in range(31)]
DMA_CHUNKS = [(0, 2), (2, 2), (4, 2), (6, 2)] + [
    (8 + 4 * b, 4) for b in range(14)
]

_CACHE = {}


def _build():
    from concourse import bacc, mybir, tile
    from concourse.tile_rust import add_dep_helper

    F32 = mybir.dt.float32
    BF16 = mybir.dt.bfloat16
    FP8 = mybir.dt.float8e3
    SIG = mybir.ActivationFunctionType.Sigmoid
    nc = bacc.Bacc(
        "TRN2", target_bir_lowering=False, debug=False, num_devices=N_CORES
    )
    x_d = nc.dram_tensor("inputs", [128, 2 * A], FP8, kind="ExternalInput")
    mps_d = nc.dram_tensor(
        "mps", [128, A * 2 * FO], FP8, kind="ExternalInput"
    )
    out_d = nc.dram_tensor("out", [C, C], BF16, kind="ExternalOutput")

    with tile.TileContext(nc) as tc:
        with (
            # head/tail chunks get dedicated buffers; the 2 MB mid-stream
            # chunks recycle through 3 buffers (WAR on the last phase-1
            # read re-gates dispatch, capping outstanding transfers at ~2)
            tc.tile_pool(name="mps2", bufs=4) as mps2,
            tc.tile_pool(name="mps4", bufs=14) as mps4,
            tc.tile_pool(name="small", bufs=1) as small,
            tc.tile_pool(name="cw_pool", bufs=8) as cw_pool,
            tc.tile_pool(name="env_pool", bufs=4) as env_pool,
            tc.tile_pool(name="ps_mat", bufs=4, space="PSUM") as ps_mat,
            tc.tile_pool(name="ps_chain", bufs=3, space="PSUM") as ps_chain,
        ):
            seg_of = {}
            for si, (g0, w) in enumerate(SEGS):
                for g in range(g0, g0 + w):
                    seg_of[g] = si

            # all input DMAs dispatch up front on the sync HWDGE ring
            x_sb = small.tile([128, 2 * A], FP8)
            nc.sync.dma_start(x_sb[:], x_d[:])
            chunk_views = []  # (c0, cw, view)
            chunk_dmas = []
            for c0, cw in DMA_CHUNKS:
                pool = {2: mps2, 4: mps4}[cw]
                t = pool.tile([128, cw * 2 * FO], FP8, tag=f"mps{cw}")
                dma = nc.sync.dma_start(
                    t[:], mps_d[:, c0 * 2 * FO : (c0 + cw) * 2 * FO]
                )
                if len(chunk_dmas) >= 6:
                    # cap outstanding transfers: chunk k dispatches only
                    # after chunk k-6 has fully landed.  Fewer outstanding
                    # transfers loses stream rate (serial dispatch path);
                    # more spreads completions late (packet round-robin
                    # across the queue) — 6 measured best.
                    add_dep_helper(
                        dma.ins,
                        chunk_dmas[-6].ins,
                        sync=True,
                        reason="limit outstanding DMA to 6",
                    )
                chunk_dmas.append(dma)
                chunk_views.append(
                    (
                        c0,
                        cw,
                        t[:].rearrange(
                            "e (gi eh c p) -> e gi eh c p",
                            gi=cw,
                            eh=2,
                            c=8,
                            p=128,
                        ),
                    )
                )

            def seg_view(g0, w):
                # slice the covering DMA chunk down to this segment
                for c0, cw, v in chunk_views:
                    if c0 <= g0 and g0 + w <= c0 + cw:
                        return v[:, g0 - c0 : g0 - c0 + w]
                raise AssertionError("segment not covered by a chunk")

            envs = [None]
            cvs = {}

            def chain_step(g, after=None):
                si = seg_of[g]
                cv, gi = cvs[si], g - SEGS[si][0]
                ps_g = ps_chain.tile([C, C], F32, tag="cps")
                pe = nc.tensor.matmul(
                    ps_g[:],
                    cv[:, gi, :],
                    envs[0],
                    start=True,
                    stop=True,
                )
                if after is not None:
                    # pin the chain matmul after the current agent block so
                    # the scheduler cannot bunch chain steps early either
                    add_dep_helper(
                        pe.ins,
                        after.ins,
                        sync=True,
                        reason="pace chain step to its agent slot",
                    )
                env2 = env_pool.tile([C, C], BF16, tag="env")
                # operands carry 256x fp8 scales (mps x128, x x2); the
                # descale folds into the sigmoid's scale.  Step 1's moving
                # operand is agent 0's raw 256x slab, hence 1/65536.
                nc.scalar.activation(
                    env2[:],
                    ps_g[:],
                    SIG,
                    scale=1.0 / 65536.0 if g == 1 else 1.0 / 256.0,
                )
                envs[0] = env2[:]
                return pe

            next_chain = 1
            last_chain = [None]
            slot = [0]
            last_burst = [-10]

            for si, (g0, w) in enumerate(SEGS):
                psum_b = ps_mat.tile([128, 8 * w], F32, tag="psa")
                tvs = seg_view(g0, w)
                for gi in range(w):
                    g = g0 + gi
                    last_mm = None
                    for c in range(8):
                        for eh in range(2):
                            mm = nc.tensor.matmul(
                                psum_b[:, gi * 8 + c : gi * 8 + c + 1],
                                tvs[:, gi, eh, c, :],
                                x_sb[:, eh * A + g : eh * A + g + 1],
                                start=(eh == 0),
                                stop=(eh == 1),
                            )
                            last_mm = mm
                            if last_chain[0] is not None:
                                # pin EVERY phase-1 matmul after the latest
                                # chain matmul (sync=True: no_sync edges
                                # are ignored for ordering) so phase-1
                                # cannot run ahead of the chain; with the
                                # reverse pin in chain_step the PE stream
                                # strictly alternates [agent block][chain
                                # step] and each sigmoid retires while the
                                # next agent block streams
                                add_dep_helper(
                                    mm.ins,
                                    last_chain[0].ins,
                                    sync=True,
                                    reason="interleave chain with phase-1",
                                )
                    # one chain step per agent block: the chain MM rides
                    # in the block's PE pipeline shadow.  A second step in
                    # one slot stalls the next block a full sigmoid
                    # round-trip, so catch-up bursts are rate-limited.
                    # elig = g - 2: chain t fires two block-cycles after
                    # its segment's relayout began, so the 4 serial DVE
                    # copies (~0.96 us end-to-end) are never on the
                    # chain's critical path.
                    slot[0] += 1
                    # head: 1-agent lag (the early CASTs retire during DMA
                    # waits, so the tighter slot is stall-free and starts
                    # the chain one block earlier); steady state: 2-agent
                    # lag keeps the per-segment relayout off the chain path
                    elig = g - 1 if g < 8 else g - 2
                    backlog = elig - next_chain
                    cap = 1
                    if backlog > 4 and slot[0] - last_burst[0] >= 6:
                        cap = 2
                        last_burst[0] = slot[0]
                    k = 0
                    while next_chain < elig and k < cap:
                        last_chain[0] = chain_step(next_chain, after=last_mm)
                        next_chain += 1
                        k += 1
                # psum[32*jh + k, gi*8 + c] -> cw[k, gi*32 + jh*8 + c]:
                # one cross-quadrant DVE copy per psum quadrant jh.
                cw = cw_pool.tile([C, 32 * w], BF16, tag="cw", name=f"cw{si}")
                cwv = cw[:].rearrange(
                    "k (gi jh c) -> k gi jh c", gi=w, jh=4, c=8
                )
                for jh in range(4):
                    nc.vector.tensor_copy(
                        cwv[:, :, jh, :],
                        psum_b[32 * jh : 32 * jh + 32, :].rearrange(
                            "k (gi c) -> k gi c", gi=w, c=8
                        ),
                    )
                cvs[si] = cw[:].rearrange("k (gi j) -> k gi j", gi=w, j=32)
                if si == 0:
                    # agent 0's slab is packed pre-transposed host-side:
                    # it IS env0 (times 256), read directly as the moving
                    # operand of chain step 1.
                    envs[0] = cvs[0][:, 0, :]
            while next_chain < A:
                chain_step(next_chain)
                next_chain += 1

            # ship the final 32x32 env; host takes the trace.  Issued on
            # the ACT engine's HWDGE ring: its queue is idle right after
            # the last sigmoid, skipping a cross-engine sem hop.
            nc.scalar.dma_start(out_d[:], envs[0])

    nc.compile()
    return nc


def get_nc():
    if "nc" not in _CACHE:
        _CACHE["nc"] = _build()
    return _CACHE["nc"]


def make_in_maps(inputs, mps):
    import ml_dtypes

    FP8 = ml_dtypes.float8_e3m4
    # power-of-two scales put both operands mid-range in e3m4 (max 15.5):
    # x ~ N(0,1) * 2, mps ~ N(0, 0.0156^2) * 128.  Combined 256x descales
    # on-device via the chain sigmoid's scale parameter.
    x = (np.asarray(inputs, dtype=np.float32) * 2.0).astype(FP8)
    mps = np.asarray(mps, dtype=np.float32).reshape(A, E, O, FO)
    # x packed as [e_low, (e_chunk, agent)]
    x_pack = np.ascontiguousarray(
        x.reshape(A, 2, 128).transpose(2, 1, 0).reshape(128, 2 * A)
    )
    # F_idx[c, p] = k*32 + j with k = p%32, j = 8*(p//32) + c: weight column
    # (gi, eh, c) reads the contiguous run packed[(gi*2+eh)*1024 + c*128 :
    # +128] and psum partitions come out as 32*jh + k (chain-quadrant
    # layout).  Agent 0 swaps k and j (transposed slab = env0 directly).
    p = np.arange(128)
    c = np.arange(8)[:, None]
    F_idx = ((p % 32) * 32 + (p // 32) * 8 + c).reshape(-1)  # (1024,)
    F_idx0 = (((p // 32) * 8 + c) * 32 + (p % 32)).reshape(-1)
    in_maps = []
    for ci in range(N_CORES):
        m = (mps[:, :, ci, :] * 128.0).astype(FP8)  # (A, E, FO)
        m = m[:, :, F_idx]  # permute f so weight columns are contiguous
        m[0] = (mps[0, :, ci, :] * 128.0).astype(FP8)[:, F_idx0]
        # -> [e_low(128), agent, e_chunk, FO]  (one contiguous run per
        #    chunk: both e-halves of its agents)
        m = m.reshape(A, 2, 128, FO).transpose(2, 0, 1, 3)
        in_maps.append(
            {
                "inputs": x_pack,
                "mps": np.ascontiguousarray(m).reshape(128, A * 2 * FO),
            }
        )
    return in_maps


def kernel(inputs, mps):
    from concourse.bass_utils import run_bass_kernel_spmd

    nc = get_nc()
    in_maps = make_in_maps(inputs, mps)
    try:
        res = run_bass_kernel_spmd(nc, in_maps, core_ids=list(range(N_CORES)))
    except Exception:
        # rare transient NRT failures; one retry
        res = run_bass_kernel_spmd(nc, in_maps, core_ids=list(range(N_CORES)))
    return np.array(
        [
            np.trace(res.results[ci]["out"].astype(np.float32))
            for ci in range(N_CORES)
        ],
        dtype=np.float32,
    )
